# revision 10
# baseline (speedup 1.0000x reference)
"""GQA attention (B=1, L=2048, D=2048, H=32, KV=8, HD=64) + RoPE + causal mask,
tensor-parallel over heads across 8 TRN2 NeuronCores.

Core i owns KV head i and Q heads 4i..4i+3. Each core computes
partial_i = O_i @ wo_i; the host sums the 8 bf16 partials.

Pipeline (per core):
  - x^T streams in via SWDGE DMA-cast (fp32 HBM -> bf16 SBUF, no engine cast),
    while K/V and Q[0:1024] projections accumulate in PSUM behind the stream.
  - RoPE runs on DVE (mults read PSUM directly; combines are cross-quadrant
    SBUF ops). Softmax-over-keys uses a ones-column in the V stationary.
  - Attention runs in 512-query stripes; the two heads of a pair issue
    back-to-back K=64 matmuls (row-tiled halves of the PE array) into one
    2-bank S psum, a single fused Exp covers both heads, and V matmuls
    accumulate O^T + denominator. Causal masking: block-granular wedge trim
    plus one lower-triangular multiplier on diagonal blocks.
  - Q[1024:2048] projection and the wo matmuls are drip-fed between S and O
    matmuls so the PE stays busy while ACT computes Exp.
"""

import numpy as np

try:
    import concourse  # noqa: F401
except ImportError:
    import sys as _sys
    for _p in ("/opt/trn_rl_repo", "/root/.axon_site/_ro/trn_rl_repo"):
        if _p not in _sys.path:
            _sys.path.insert(0, _p)

B, L, D = 1, 2048, 2048
H, KV, HD = 32, 8, 64
NCORES = 8
P = 128
KT = D // P          # 16 contraction tiles
LB = L // P          # 16 key blocks
NS = 4               # query stripes of 512


def _build_nc(reps: int = 1):
    import concourse.mybir as mybir
    import concourse.tile as tile
    from concourse import bacc
    from concourse.bass import ts, ds
    from concourse.masks import make_identity

    f32 = mybir.dt.float32
    bf16 = mybir.dt.bfloat16
    EXP = mybir.ActivationFunctionType.Exp
    ADD = mybir.AluOpType.add
    SUB = mybir.AluOpType.subtract
    MULT = mybir.AluOpType.mult

    nc = bacc.Bacc(None, target_bir_lowering=False, debug=False)

    x_t = nc.declare_dram_parameter("x_t", [D, L], f32, isOutput=False)
    wq_p = nc.declare_dram_parameter("wq_p", [D, 256], f32, isOutput=False)
    wkv = nc.declare_dram_parameter("wkv", [D, 128], f32, isOutput=False)
    wo_p = nc.declare_dram_parameter("wo_p", [256, D], f32, isOutput=False)
    cos2 = nc.declare_dram_parameter("cos2", [64, L], f32, isOutput=False)
    sin2 = nc.declare_dram_parameter("sin2", [64, L], f32, isOutput=False)
    part = nc.declare_dram_parameter("part", [L, D], bf16, isOutput=True)

    with tile.TileContext(nc) as tc:
        with tc.tile_pool(name="persist", bufs=1) as pp:
          for _rep in range(reps):
            # ---------- persistent SBUF ----------
            xt_sb = pp.tile([P, KT, L], bf16, tag="xt")
            wq_sb = pp.tile([P, KT, 256], bf16, tag="wq")
            kv_sb = pp.tile([P, KT, 128], bf16, tag="kv")
            wo_sb = pp.tile([P, 2, L], bf16, tag="wo")
            qt_sb = pp.tile([P, 2, L], bf16, tag="qt")     # [64*hh+dd, pb, q]
            kt_sb = pp.tile([P, L], bf16, tag="kt")        # rows 64:128 dup
            vn_sb = pp.tile([P, LB, 65], bf16, tag="vn")   # [k%128, kb, dd|1]
            ot_sb = pp.tile([P, 2, L], bf16, tag="ot")
            cos_sb = pp.tile([64, L], f32, tag="cos")
            sin_sb = pp.tile([64, L], f32, tag="sin")
            em_sb = pp.tile([P, P], bf16, tag="em")        # tril(k<=q) 0/1
            ident = pp.tile([64, 64], bf16, tag="ident")
            warm = pp.tile([1, 16], f32, tag="warm")

            make_identity(nc, ident[:])
            # em[k, q] = 1 if k <= q else 0  (same for every diagonal block)
            nc.gpsimd.memset(em_sb[:], 1.0)
            nc.gpsimd.affine_select(
                out=em_sb[:], in_=em_sb[:],
                compare_op=mybir.AluOpType.is_ge, fill=0.0,
                base=0, channel_multiplier=-1, pattern=[[1, P]],
            )
            # preload the ACT exp table off the critical path
            nc.vector.memset(warm[:], 0.0)
            nc.scalar.activation(warm[:], warm[:], EXP)
            nc.vector.memset(vn_sb[:, :, 64:65], 1.0)
            nc.sync.dma_start(cos_sb[:], cos2[:, :])
            nc.sync.dma_start(sin_sb[:], sin2[:, :])

            # ---------- phase A: stream x (HWDGE fp32 + engine casts) ------
            # wq/wkv arrive early via SWDGE dma-cast; x tiles alternate the
            # two HWDGE rings and get cast to bf16 on DVE/ACT (idle here).
            # wo is queued on the HWDGE rings AFTER x so it lands ~when the
            # wo matmuls start, without competing with the x stream.
            with (
                tc.tile_pool(name="xstage", bufs=3) as xst,
                tc.tile_pool(name="rope", bufs=1) as rtp,
                tc.tile_pool(name="psum_q0", bufs=2, space="PSUM") as ps_q0,
            ):
                kv_f = pp.tile([P, KT, 128], f32, tag="kv_f")
                wq_f = pp.tile([P, KT, 256], f32, tag="wq_f")
                nc.sync.dma_start(
                    kv_f[:], wkv.ap().rearrange("(t p) n -> p t n", p=P))
                nc.scalar.dma_start(
                    wq_f[:], wq_p.ap().rearrange("(t p) n -> p t n", p=P))
                nc.scalar.copy(kv_sb[:], kv_f[:])
                nc.scalar.copy(wq_sb[:], wq_f[:])

                vt_tmp = pp.tile([64, L], bf16, tag="vt")
                q0ps = [ps_q0.tile([P, 2, 512], f32, tag="q0_ps", name=f"q0ps{pb}")
                        for pb in range(2)]
                with tc.tile_pool(name="psum_kv", bufs=1, space="PSUM") as ps_kv:
                    kvps = ps_kv.tile([P, 4, 512], f32, tag="kv_ps")
                    for t in range(KT):
                        xf = xst.tile([P, L], f32, tag="x_f")
                        (nc.sync if t % 2 == 0 else nc.scalar).dma_start(
                            xf[:], x_t[ts(t, P), :])
                        if t % 2 == 0:
                            nc.vector.tensor_copy(xt_sb[:, t, :], xf[:])
                        else:
                            nc.scalar.copy(xt_sb[:, t, :], xf[:])
                        for jj in range(4):
                            nc.tensor.matmul(
                                kvps[:, jj, :], kv_sb[:, t, :],
                                xt_sb[:, t, ts(jj, 512)],
                                start=(t == 0), stop=(t == KT - 1),
                            )
                        for pb in range(2):
                            for hf in range(2):
                                nc.tensor.matmul(
                                    q0ps[pb][:, hf, :],
                                    wq_sb[:, t, ts(pb, P)],
                                    xt_sb[:, t, ts(hf, 512)],
                                    start=(t == 0), stop=(t == KT - 1),
                                )
                    # wo: 2 fp32 tiles behind x on the HWDGE rings + DVE cast
                    wo_f0 = xst.tile([P, L], f32, tag="x_f", name="wo_f0")
                    wo_f1 = xst.tile([P, L], f32, tag="x_f", name="wo_f1")
                    nc.sync.dma_start(
                        wo_f0[:], wo_p.ap()[0:128, :])
                    nc.scalar.dma_start(
                        wo_f1[:], wo_p.ap()[128:256, :])
                    nc.vector.tensor_copy(wo_sb[:, 0, :], wo_f0[:])
                    nc.vector.tensor_copy(wo_sb[:, 1, :], wo_f1[:])

                    # K RoPE in two halves (rows 0:32 real, 32:64 imag)
                    kvv = kvps[:, :, :].rearrange("p a b -> p (a b)")
                    for jh in range(2):
                        ksl = ds(jh * 1024, 1024)
                        t_rc = rtp.tile([32, 1024], f32, tag="t_rc")
                        t_is = rtp.tile([32, 1024], f32, tag="t_is")
                        t_rs = rtp.tile([32, 1024], f32, tag="t_rs")
                        t_ic = rtp.tile([32, 1024], f32, tag="t_ic")
                        nc.vector.tensor_tensor(t_rc[:], kvv[0:32, ksl], cos_sb[0:32, ksl], MULT)
                        nc.vector.tensor_tensor(t_is[:], kvv[32:64, ksl], sin_sb[32:64, ksl], MULT)
                        nc.vector.tensor_tensor(t_rs[:], kvv[0:32, ksl], sin_sb[0:32, ksl], MULT)
                        nc.vector.tensor_tensor(t_ic[:], kvv[32:64, ksl], cos_sb[32:64, ksl], MULT)
                        nc.vector.tensor_tensor(kt_sb[0:32, ksl], t_rc[:], t_is[:], SUB)
                        nc.vector.tensor_tensor(kt_sb[32:64, ksl], t_rs[:], t_ic[:], ADD)
                    # V^T (psum rows 64:128) -> bf16 staging for PE transpose
                    nc.scalar.copy(vt_tmp[:], kvv[64:128, :])
                    # duplicate K^T into partitions 64:128 for head-B S tiles
                    nc.sync.dma_start(kt_sb[64:128, :], kt_sb[0:64, :])

                # V natural via PE transpose (kv banks just freed)
                with tc.tile_pool(name="psum_vt", bufs=2, space="PSUM") as ps_vt:
                    for kb in range(LB):
                        vps = ps_vt.tile([P, 64], bf16, tag="v_ps")
                        nc.tensor.matmul(
                            vps[:], vt_tmp[:, ts(kb, P)], ident[:],
                            start=True, stop=True, is_transpose=True,
                        )
                        nc.scalar.copy(vn_sb[:, kb, 0:64], vps[:])

                    # Q0 RoPE per pb: psum rows 0:32 A-r, 32:64 B-r,
                    # 64:96 A-i, 96:128 B-i; cos/sin rows 0:64 = freqs dup'd.
                    for pb in range(2):
                        qv = q0ps[pb][:, :, :].rearrange("p a b -> p (a b)")
                        m1 = rtp.tile([64, 1024], f32, tag="m1")
                        m2 = rtp.tile([64, 1024], f32, tag="m2")
                        m3 = rtp.tile([64, 1024], f32, tag="m3")
                        m4 = rtp.tile([64, 1024], f32, tag="m4")
                        qsl = ds(0, 1024)
                        nc.vector.tensor_tensor(m1[:], qv[0:64, :], cos_sb[:, qsl], MULT)
                        nc.vector.tensor_tensor(m2[:], qv[64:128, :], sin_sb[:, qsl], MULT)
                        nc.vector.tensor_tensor(m3[:], qv[0:64, :], sin_sb[:, qsl], MULT)
                        nc.vector.tensor_tensor(m4[:], qv[64:128, :], cos_sb[:, qsl], MULT)
                        nc.vector.tensor_tensor(qt_sb[0:32, pb, qsl], m1[0:32, :], m2[0:32, :], SUB)
                        nc.vector.tensor_tensor(qt_sb[64:96, pb, qsl], m1[32:64, :], m2[32:64, :], SUB)
                        nc.vector.tensor_tensor(qt_sb[32:64, pb, qsl], m3[0:32, :], m4[0:32, :], ADD)
                        nc.vector.tensor_tensor(qt_sb[96:128, pb, qsl], m3[32:64, :], m4[32:64, :], ADD)

            # ---------- phase B: Q1 at the seam, attention stripes, wo drip --
            with (
                tc.tile_pool(name="attn_et", bufs=3) as etp,
                tc.tile_pool(name="norm_sb", bufs=2) as nsb,
                tc.tile_pool(name="out_sb", bufs=4) as osb,
                tc.tile_pool(name="rope1", bufs=1) as rt1,
                tc.tile_pool(name="psum_s", bufs=2, space="PSUM") as ps_s,
                tc.tile_pool(name="psum_o", bufs=1, space="PSUM") as ps_o,
                tc.tile_pool(name="psum_m", bufs=1, space="PSUM") as ps_m,
            ):
                def q1_rope(q1ps, pb):
                    qv = q1ps[:, :, :].rearrange("p a b -> p (a b)")
                    m1 = rt1.tile([64, 1024], f32, tag="n1")
                    m2 = rt1.tile([64, 1024], f32, tag="n2")
                    m3 = rt1.tile([64, 1024], f32, tag="n3")
                    m4 = rt1.tile([64, 1024], f32, tag="n4")
                    qsl = ds(1024, 1024)
                    nc.vector.tensor_tensor(m1[:], qv[0:64, :], cos_sb[:, qsl], MULT)
                    nc.vector.tensor_tensor(m2[:], qv[64:128, :], sin_sb[:, qsl], MULT)
                    nc.vector.tensor_tensor(m3[:], qv[0:64, :], sin_sb[:, qsl], MULT)
                    nc.vector.tensor_tensor(m4[:], qv[64:128, :], cos_sb[:, qsl], MULT)
                    nc.vector.tensor_tensor(qt_sb[0:32, pb, qsl], m1[0:32, :], m2[0:32, :], SUB)
                    nc.vector.tensor_tensor(qt_sb[64:96, pb, qsl], m1[32:64, :], m2[32:64, :], SUB)
                    nc.vector.tensor_tensor(qt_sb[32:64, pb, qsl], m3[0:32, :], m4[0:32, :], ADD)
                    nc.vector.tensor_tensor(qt_sb[96:128, pb, qsl], m3[32:64, :], m4[32:64, :], ADD)

                # Q1 projection runs at the seam (PE idle during K/Q0 RoPE);
                # its RoPE overlaps attention stripe 0 on DVE.
                for pb in range(2):
                    q1ps = ps_m.tile([P, 2, 512], f32, tag="m_ps",
                                     name=f"q1ps{pb}")
                    for t in range(KT):
                        for hf in range(2):
                            nc.tensor.matmul(
                                q1ps[:, hf, :],
                                wq_sb[:, t, ts(pb, P)],
                                xt_sb[:, t, ds(1024 + hf * 512, 512)],
                                start=(t == 0), stop=(t == KT - 1),
                            )
                    q1_rope(q1ps, pb)

                def wo_unit(lq, n, evac=None):
                    wps = ps_m.tile([P, 2, 512], f32, tag="m_ps")
                    for hf in range(2):
                        for t in range(2):
                            nc.tensor.matmul(
                                wps[:, hf, :],
                                ot_sb[:, t, ts(lq, P)],
                                wo_sb[:, t, ds(n * 1024 + hf * 512, 512)],
                                start=(t == 0), stop=(t == 1),
                            )
                    ob = osb.tile([P, 1024], bf16, tag="o_sb")
                    if evac is None:
                        nc.vector.tensor_copy(ob[:], wps[:, :, :].rearrange("p a b -> p (a b)"))
                    else:
                        evac(ob[:], wps[:, :, :].rearrange("p a b -> p (a b)"))
                    deng = nc.sync if n % 2 == 0 else nc.scalar
                    deng.dma_start(part[ts(lq, P), ts(n, 1024)], ob[:])

                # drip queue of closures popped between S and O matmuls
                drip = []

                def pop_drip(k=1):
                    for _ in range(k):
                        if drip:
                            drip.pop(0)()

                for s in range(NS):
                    if s >= 1:
                        drip += [(lambda lq=lq, n=n: wo_unit(lq, n))
                                 for lq in range(4 * (s - 1), 4 * s)
                                 for n in range(2)]
                    qs = 512 * s
                    nkb = 4 * (s + 1)
                    for pb in range(2):
                        ops = ps_o.tile([65, 2, 512], f32, tag="o_ps")
                        for kb in range(nkb):
                            c0 = max(0, kb - 4 * s) * P
                            w = 512 - c0
                            sps = ps_s.tile([P, 2, 512], f32, tag="s_ps")
                            for hh in range(2):
                                nc.tensor.matmul(
                                    sps[:, hh, c0:512],
                                    kt_sb[ds(64 * hh, 64), ts(kb, P)],
                                    qt_sb[ds(64 * hh, 64), pb, ds(qs + c0, w)],
                                    start=True, stop=True,
                                )
                            et = etp.tile([P, 2, 512], bf16, tag="e_t")
                            nc.scalar.activation(
                                et[:, :, c0:512], sps[:, :, c0:512], EXP,
                                scale=0.125,
                            )
                            if kb >= 4 * s:
                                for hh in range(2):
                                    nc.vector.tensor_tensor(
                                        et[:, hh, ds(c0, P)], et[:, hh, ds(c0, P)],
                                        em_sb[:], MULT,
                                    )
                            # keep PE fed while ACT computes the exp
                            pop_drip(1)
                            for hh in range(2):
                                nc.tensor.matmul(
                                    ops[:, hh, c0:512],
                                    vn_sb[:, kb, :],
                                    et[:, hh, ds(c0, w)],
                                    start=(kb == 0), stop=(kb == nkb - 1),
                                )
                        # normalize straight out of PSUM: ot = o * (1/denom)
                        for hh in range(2):
                            rr = nsb.tile([1, 512], f32, tag="r_r")
                            nc.vector.tensor_copy(rr[:], ops[64:65, hh, :])
                            rq = nsb.tile([1, 512], f32, tag="r_q")
                            nc.vector.reciprocal_approx_fast(rq[:], rr[:])
                            rb = nsb.tile([64, 512], f32, tag="r_b")
                            nc.gpsimd.partition_broadcast(rb[:], rq[:])
                            nc.vector.tensor_tensor(
                                ot_sb[ds(64 * hh, 64), pb, ds(qs, 512)],
                                ops[0:64, hh, :], rb[:], MULT,
                            )
                # tail: last stripe's wo + anything left in the queue
                while drip:
                    pop_drip(1)
                for lq in range(12, 16):
                    for n in range(2):
                        wo_unit(lq, n, evac=nc.scalar.copy)

    nc.compile()
    return nc


_NC_CACHE = None


def _get_nc():
    global _NC_CACHE
    if _NC_CACHE is None:
        _NC_CACHE = _build_nc()
    return _NC_CACHE


def _shard_inputs(x, wq, wk, wv, wo, freqs_cos, freqs_sin, mask):
    """Host-side shard prep: pure layout/indexing transforms, no arithmetic."""
    f = np.float32
    perm = np.empty(64, np.int64)
    perm[:32] = 2 * np.arange(32)
    perm[32:] = 2 * np.arange(32) + 1

    x_t = np.ascontiguousarray(np.asarray(x, f).reshape(L, D).T)
    cosT = np.ascontiguousarray(np.asarray(freqs_cos, f).T)
    sinT = np.ascontiguousarray(np.asarray(freqs_sin, f).T)
    cos2 = np.ascontiguousarray(np.concatenate([cosT, cosT], 0))
    sin2 = np.ascontiguousarray(np.concatenate([sinT, sinT], 0))

    wq = np.asarray(wq, f)
    wk = np.asarray(wk, f)
    wv = np.asarray(wv, f)
    wo = np.asarray(wo, f)

    in_maps = []
    for i in range(NCORES):
        wq_i = wq[:, 4 * i * 64:(4 * i + 4) * 64]
        cols = []
        for pb in range(2):
            A = wq_i[:, (2 * pb) * 64:(2 * pb + 1) * 64][:, perm]
            Bc = wq_i[:, (2 * pb + 1) * 64:(2 * pb + 2) * 64][:, perm]
            cols.append(np.concatenate([A[:, :32], Bc[:, :32], A[:, 32:], Bc[:, 32:]], 1))
        wq_p = np.ascontiguousarray(np.concatenate(cols, 1))
        wk_p = wk[:, i * 64:(i + 1) * 64][:, perm]
        wv_i = wv[:, i * 64:(i + 1) * 64]
        wkv = np.ascontiguousarray(np.concatenate([wk_p, wv_i], 1))
        wo_i = np.ascontiguousarray(wo[4 * i * 64:(4 * i + 4) * 64, :])
        in_maps.append({
            "x_t": x_t, "wq_p": wq_p, "wkv": wkv, "wo_p": wo_i,
            "cos2": cos2, "sin2": sin2,
        })
    return in_maps


_last_results = None


def kernel(x, wq, wk, wv, wo, freqs_cos, freqs_sin, mask):
    global _last_results
    from concourse.bass_utils import run_bass_kernel_spmd

    nc = _get_nc()
    in_maps = _shard_inputs(x, wq, wk, wv, wo, freqs_cos, freqs_sin, mask)
    res = run_bass_kernel_spmd(nc, in_maps, core_ids=list(range(NCORES)))
    _last_results = res
    out = np.zeros((L, D), np.float32)
    for i in range(NCORES):
        out += np.asarray(res.results[i]["part"]).astype(np.float32)
    return out.reshape(B, L, D)


# revision 13
# speedup vs baseline: 1.2866x; 1.2866x over previous
"""GQA attention (B=1, L=2048, D=2048, H=32, KV=8, HD=64) + RoPE + causal mask,
tensor-parallel over heads across 8 TRN2 NeuronCores.

Core i owns KV head i and Q heads 4i..4i+3. Each core computes
partial_i = O_i @ wo_i; the host sums the 8 bf16 partials.

Pipeline (per core):
  - x^T streams in via SWDGE DMA-cast (fp32 HBM -> bf16 SBUF, no engine cast),
    while K/V and Q[0:1024] projections accumulate in PSUM behind the stream.
  - RoPE runs on DVE (mults read PSUM directly; combines are cross-quadrant
    SBUF ops). Softmax-over-keys uses a ones-column in the V stationary.
  - Attention runs in 512-query stripes; the two heads of a pair issue
    back-to-back K=64 matmuls (row-tiled halves of the PE array) into one
    2-bank S psum, a single fused Exp covers both heads, and V matmuls
    accumulate O^T + denominator. Causal masking: block-granular wedge trim
    plus one lower-triangular multiplier on diagonal blocks.
  - Q[1024:2048] projection and the wo matmuls are drip-fed between S and O
    matmuls so the PE stays busy while ACT computes Exp.
"""

import numpy as np

try:
    import concourse  # noqa: F401
except ImportError:
    import sys as _sys
    for _p in ("/opt/trn_rl_repo", "/root/.axon_site/_ro/trn_rl_repo"):
        if _p not in _sys.path:
            _sys.path.insert(0, _p)

B, L, D = 1, 2048, 2048
H, KV, HD = 32, 8, 64
NCORES = 8
P = 128
KT = D // P          # 16 contraction tiles
LB = L // P          # 16 key blocks
NS = 4               # query stripes of 512


def _build_nc(reps: int = 1):
    import concourse.mybir as mybir
    import concourse.tile as tile
    from concourse import bacc
    from concourse.bass import ts, ds
    from concourse.masks import make_identity

    f32 = mybir.dt.float32
    bf16 = mybir.dt.bfloat16
    EXP = mybir.ActivationFunctionType.Exp
    ADD = mybir.AluOpType.add
    SUB = mybir.AluOpType.subtract
    MULT = mybir.AluOpType.mult

    nc = bacc.Bacc(None, target_bir_lowering=False, debug=False)

    x_t = nc.declare_dram_parameter("x_t", [D, L], f32, isOutput=False)
    wq_p = nc.declare_dram_parameter("wq_p", [D, 256], f32, isOutput=False)
    wkv = nc.declare_dram_parameter("wkv", [D, 128], f32, isOutput=False)
    wo_p = nc.declare_dram_parameter("wo_p", [256, D], f32, isOutput=False)
    cos2 = nc.declare_dram_parameter("cos2", [64, L], f32, isOutput=False)
    sin2 = nc.declare_dram_parameter("sin2", [64, L], f32, isOutput=False)
    part = nc.declare_dram_parameter("part", [L, D], bf16, isOutput=True)

    with tile.TileContext(nc) as tc:
        with tc.tile_pool(name="persist", bufs=1) as pp:
          for _rep in range(reps):
            # ---------- persistent SBUF ----------
            xt_sb = pp.tile([P, KT, L], bf16, tag="xt")
            wq_sb = pp.tile([P, KT, 256], bf16, tag="wq")
            kv_sb = pp.tile([P, KT, 128], bf16, tag="kv")
            wo_sb = pp.tile([P, 2, L], bf16, tag="wo")
            qt_sb = pp.tile([P, 2, L], bf16, tag="qt")     # [64*hh+dd, pb, q]
            kt_sb = pp.tile([P, L], bf16, tag="kt")        # rows 64:128 dup
            vn_sb = pp.tile([P, LB, 65], bf16, tag="vn")   # [k%128, kb, dd|1]
            ot_sb = pp.tile([P, 2, L], bf16, tag="ot")
            cos_sb = pp.tile([64, L], f32, tag="cos")
            sin_sb = pp.tile([64, L], f32, tag="sin")
            ident = pp.tile([64, 64], bf16, tag="ident")
            negi = pp.tile([P, P], bf16, tag="negi")      # -1e9 on the diagonal
            ustr = pp.tile([P, P], bf16, tag="ustr")      # 1 where k > q
            warm = pp.tile([1, 16], f32, tag="warm")

            make_identity(nc, ident[:])
            # negI @ ustrict adds -1e9 to the strictly-upper (k > q) region of
            # a diagonal S block, so the causal mask rides the PE instead of a
            # DVE multiply after the exp.
            nc.gpsimd.memset(negi[:], 0.0)
            nc.gpsimd.affine_select(
                out=negi[:], in_=negi[:],
                compare_op=mybir.AluOpType.not_equal, fill=-1e9,
                base=0, channel_multiplier=1, pattern=[[-1, P]],
            )
            nc.gpsimd.memset(ustr[:], 1.0)
            nc.gpsimd.affine_select(
                out=ustr[:], in_=ustr[:],
                compare_op=mybir.AluOpType.is_gt, fill=0.0,
                base=0, channel_multiplier=1, pattern=[[-1, P]],
            )
            # preload the ACT exp table off the critical path
            nc.vector.memset(warm[:], 0.0)
            nc.scalar.activation(warm[:], warm[:], EXP)
            nc.vector.memset(vn_sb[:, :, 64:65], 1.0)
            nc.sync.dma_start(cos_sb[:], cos2[:, :])
            nc.sync.dma_start(sin_sb[:], sin2[:, :])

            # ---------- phase A: stream x (HWDGE fp32 + engine casts) ------
            # wq/wkv arrive early via SWDGE dma-cast; x tiles alternate the
            # two HWDGE rings and get cast to bf16 on DVE/ACT (idle here).
            # wo is queued on the HWDGE rings AFTER x so it lands ~when the
            # wo matmuls start, without competing with the x stream.
            with (
                tc.tile_pool(name="xstage", bufs=3) as xst,
                tc.tile_pool(name="rope", bufs=1) as rtp,
                tc.tile_pool(name="psum_q0", bufs=2, space="PSUM") as ps_q0,
            ):
                kv_f = pp.tile([P, KT, 128], f32, tag="kv_f")
                wq_f = pp.tile([P, KT, 256], f32, tag="wq_f")
                nc.sync.dma_start(
                    kv_f[:], wkv.ap().rearrange("(t p) n -> p t n", p=P))
                nc.scalar.dma_start(
                    wq_f[:], wq_p.ap().rearrange("(t p) n -> p t n", p=P))
                nc.scalar.copy(kv_sb[:], kv_f[:])
                nc.scalar.copy(wq_sb[:], wq_f[:])

                vt_tmp = pp.tile([64, L], bf16, tag="vt")
                q0ps = [ps_q0.tile([P, 2, 512], f32, tag="q0_ps", name=f"q0ps{pb}")
                        for pb in range(2)]
                with tc.tile_pool(name="psum_kv", bufs=1, space="PSUM") as ps_kv:
                    kvps = ps_kv.tile([P, 4, 512], f32, tag="kv_ps")
                    for t in range(KT):
                        xf = xst.tile([P, L], f32, tag="x_f")
                        (nc.sync if t % 2 == 0 else nc.scalar).dma_start(
                            xf[:], x_t[ts(t, P), :])
                        if t % 2 == 0:
                            nc.vector.tensor_copy(xt_sb[:, t, :], xf[:])
                        else:
                            nc.scalar.copy(xt_sb[:, t, :], xf[:])
                        for jj in range(4):
                            nc.tensor.matmul(
                                kvps[:, jj, :], kv_sb[:, t, :],
                                xt_sb[:, t, ts(jj, 512)],
                                start=(t == 0), stop=(t == KT - 1),
                            )
                        for pb in range(2):
                            for hf in range(2):
                                nc.tensor.matmul(
                                    q0ps[pb][:, hf, :],
                                    wq_sb[:, t, ts(pb, P)],
                                    xt_sb[:, t, ts(hf, 512)],
                                    start=(t == 0), stop=(t == KT - 1),
                                )
                    # wo: 2 fp32 tiles behind x on the HWDGE rings + DVE cast
                    wo_f0 = xst.tile([P, L], f32, tag="x_f", name="wo_f0")
                    wo_f1 = xst.tile([P, L], f32, tag="x_f", name="wo_f1")
                    nc.sync.dma_start(
                        wo_f0[:], wo_p.ap()[0:128, :])
                    nc.scalar.dma_start(
                        wo_f1[:], wo_p.ap()[128:256, :])
                    nc.scalar.copy(wo_sb[:, 0, :], wo_f0[:])
                    nc.scalar.copy(wo_sb[:, 1, :], wo_f1[:])

                    # K RoPE in two halves (rows 0:32 real, 32:64 imag)
                    kvv = kvps[:, :, :].rearrange("p a b -> p (a b)")
                    for jh in range(2):
                        ksl = ds(jh * 1024, 1024)
                        t_rc = rtp.tile([32, 1024], f32, tag="t_rc")
                        t_is = rtp.tile([32, 1024], f32, tag="t_is")
                        t_rs = rtp.tile([32, 1024], f32, tag="t_rs")
                        t_ic = rtp.tile([32, 1024], f32, tag="t_ic")
                        nc.vector.tensor_tensor(t_rc[:], kvv[0:32, ksl], cos_sb[0:32, ksl], MULT)
                        nc.vector.tensor_tensor(t_is[:], kvv[32:64, ksl], sin_sb[32:64, ksl], MULT)
                        nc.vector.tensor_tensor(t_rs[:], kvv[0:32, ksl], sin_sb[0:32, ksl], MULT)
                        nc.vector.tensor_tensor(t_ic[:], kvv[32:64, ksl], cos_sb[32:64, ksl], MULT)
                        nc.vector.tensor_tensor(kt_sb[0:32, ksl], t_rc[:], t_is[:], SUB)
                        nc.vector.tensor_tensor(kt_sb[32:64, ksl], t_rs[:], t_ic[:], ADD)
                    # V^T (psum rows 64:128) -> bf16 staging for PE transpose
                    nc.scalar.copy(vt_tmp[:], kvv[64:128, :])
                    # duplicate K^T into partitions 64:128 for head-B S tiles
                    nc.sync.dma_start(kt_sb[64:128, :], kt_sb[0:64, :])

                # V natural via PE transpose (kv banks just freed)
                with tc.tile_pool(name="psum_vt", bufs=2, space="PSUM") as ps_vt:
                    for kb in range(LB):
                        vps = ps_vt.tile([P, 64], bf16, tag="v_ps")
                        nc.tensor.matmul(
                            vps[:], vt_tmp[:, ts(kb, P)], ident[:],
                            start=True, stop=True, is_transpose=True,
                        )
                        nc.scalar.copy(vn_sb[:, kb, 0:64], vps[:])

                    # Q0 RoPE per pb: psum rows 0:32 A-r, 32:64 B-r,
                    # 64:96 A-i, 96:128 B-i; cos/sin rows 0:64 = freqs dup'd.
                    for pb in range(2):
                        qv = q0ps[pb][:, :, :].rearrange("p a b -> p (a b)")
                        m1 = rtp.tile([64, 1024], f32, tag="m1")
                        m2 = rtp.tile([64, 1024], f32, tag="m2")
                        m3 = rtp.tile([64, 1024], f32, tag="m3")
                        m4 = rtp.tile([64, 1024], f32, tag="m4")
                        qsl = ds(0, 1024)
                        nc.vector.tensor_tensor(m1[:], qv[0:64, :], cos_sb[:, qsl], MULT)
                        nc.vector.tensor_tensor(m2[:], qv[64:128, :], sin_sb[:, qsl], MULT)
                        nc.vector.tensor_tensor(m3[:], qv[0:64, :], sin_sb[:, qsl], MULT)
                        nc.vector.tensor_tensor(m4[:], qv[64:128, :], cos_sb[:, qsl], MULT)
                        nc.vector.tensor_tensor(qt_sb[0:32, pb, qsl], m1[0:32, :], m2[0:32, :], SUB)
                        nc.vector.tensor_tensor(qt_sb[64:96, pb, qsl], m1[32:64, :], m2[32:64, :], SUB)
                        nc.vector.tensor_tensor(qt_sb[32:64, pb, qsl], m3[0:32, :], m4[0:32, :], ADD)
                        nc.vector.tensor_tensor(qt_sb[96:128, pb, qsl], m3[32:64, :], m4[32:64, :], ADD)

            # ---------- phase B: Q1 at the seam, attention stripes, wo drip --
            with (
                tc.tile_pool(name="attn_et", bufs=3) as etp,
                tc.tile_pool(name="norm_sb", bufs=1) as nsb,
                tc.tile_pool(name="out_sb", bufs=4) as osb,
                tc.tile_pool(name="rope1", bufs=1) as rt1,
                tc.tile_pool(name="psum_s", bufs=2, space="PSUM") as ps_s,
                tc.tile_pool(name="psum_o", bufs=1, space="PSUM") as ps_o,
                tc.tile_pool(name="psum_m", bufs=1, space="PSUM") as ps_m,
            ):
                def q1_rope(q1ps, pb):
                    qv = q1ps[:, :, :].rearrange("p a b -> p (a b)")
                    m1 = rt1.tile([64, 1024], f32, tag="n1")
                    m2 = rt1.tile([64, 1024], f32, tag="n2")
                    m3 = rt1.tile([64, 1024], f32, tag="n3")
                    m4 = rt1.tile([64, 1024], f32, tag="n4")
                    qsl = ds(1024, 1024)
                    nc.vector.tensor_tensor(m1[:], qv[0:64, :], cos_sb[:, qsl], MULT)
                    nc.vector.tensor_tensor(m2[:], qv[64:128, :], sin_sb[:, qsl], MULT)
                    nc.vector.tensor_tensor(m3[:], qv[0:64, :], sin_sb[:, qsl], MULT)
                    nc.vector.tensor_tensor(m4[:], qv[64:128, :], cos_sb[:, qsl], MULT)
                    nc.vector.tensor_tensor(qt_sb[0:32, pb, qsl], m1[0:32, :], m2[0:32, :], SUB)
                    nc.vector.tensor_tensor(qt_sb[64:96, pb, qsl], m1[32:64, :], m2[32:64, :], SUB)
                    nc.vector.tensor_tensor(qt_sb[32:64, pb, qsl], m3[0:32, :], m4[0:32, :], ADD)
                    nc.vector.tensor_tensor(qt_sb[96:128, pb, qsl], m3[32:64, :], m4[32:64, :], ADD)

                # Q1 projection runs at the seam (PE idle during K/Q0 RoPE);
                # its RoPE overlaps attention stripe 0 on DVE.
                for pb in range(2):
                    q1ps = ps_m.tile([P, 2, 512], f32, tag="m_ps",
                                     name=f"q1ps{pb}")
                    for t in range(KT):
                        for hf in range(2):
                            nc.tensor.matmul(
                                q1ps[:, hf, :],
                                wq_sb[:, t, ts(pb, P)],
                                xt_sb[:, t, ds(1024 + hf * 512, 512)],
                                start=(t == 0), stop=(t == KT - 1),
                            )
                    q1_rope(q1ps, pb)

                def wo_unit(lq, n, evac=None):
                    wps = ps_m.tile([P, 2, 512], f32, tag="m_ps")
                    for hf in range(2):
                        for t in range(2):
                            nc.tensor.matmul(
                                wps[:, hf, :],
                                ot_sb[:, t, ts(lq, P)],
                                wo_sb[:, t, ds(n * 1024 + hf * 512, 512)],
                                start=(t == 0), stop=(t == 1),
                            )
                    ob = osb.tile([P, 1024], bf16, tag="o_sb")
                    if evac is None:
                        nc.vector.tensor_copy(ob[:], wps[:, :, :].rearrange("p a b -> p (a b)"))
                    else:
                        evac(ob[:], wps[:, :, :].rearrange("p a b -> p (a b)"))
                    deng = nc.sync if n % 2 == 0 else nc.scalar
                    deng.dma_start(part[ts(lq, P), ts(n, 1024)], ob[:])

                # drip queue of closures popped between S and O matmuls
                drip = []

                def pop_drip(k=1):
                    for _ in range(k):
                        if drip:
                            drip.pop(0)()

                for s in range(NS):
                    if s >= 1:
                        drip += [(lambda lq=lq, n=n: wo_unit(lq, n))
                                 for lq in range(4 * (s - 1), 4 * s)
                                 for n in range(2)]
                    qs = 512 * s
                    nkb = 4 * (s + 1)
                    for pb in range(2):
                        ops = ps_o.tile([65, 2, 512], f32, tag="o_ps")
                        for kb in range(nkb):
                            c0 = max(0, kb - 4 * s) * P
                            w = 512 - c0
                            diag = kb >= 4 * s
                            sps = ps_s.tile([P, 2, 512], f32, tag="s_ps")
                            for hh in range(2):
                                nc.tensor.matmul(
                                    sps[:, hh, c0:512],
                                    kt_sb[ds(64 * hh, 64), ts(kb, P)],
                                    qt_sb[ds(64 * hh, 64), pb, ds(qs + c0, w)],
                                    start=True, stop=not diag,
                                )
                            if diag:
                                for hh in range(2):
                                    nc.tensor.matmul(
                                        sps[:, hh, c0:c0 + P],
                                        negi[:], ustr[:],
                                        start=False, stop=True,
                                    )
                            et = etp.tile([P, 2, 512], bf16, tag="e_t")
                            nc.scalar.activation(
                                et[:, :, c0:512], sps[:, :, c0:512], EXP,
                                scale=0.125,
                            )
                            # keep PE fed while ACT computes the exp
                            pop_drip(1)
                            for hh in range(2):
                                nc.tensor.matmul(
                                    ops[:, hh, c0:512],
                                    vn_sb[:, kb, :],
                                    et[:, hh, ds(c0, w)],
                                    start=(kb == 0), stop=(kb == nkb - 1),
                                )
                        # normalize: evacuate first to free the O banks,
                        # then 1/denom from the SBUF copy
                        oev = nsb.tile([65, 2, 512], f32, tag="o_ev")
                        nc.vector.tensor_copy(oev[:], ops[:])
                        rr = nsb.tile([1, 1024], f32, tag="r_r")
                        nc.vector.tensor_copy(
                            rr[:], oev[64:65, :, :].rearrange("p a b -> p (a b)"))
                        rq = nsb.tile([1, 1024], f32, tag="r_q")
                        nc.vector.reciprocal_approx_fast(rq[:], rr[:])
                        rb = nsb.tile([64, 1024], f32, tag="r_b")
                        nc.gpsimd.partition_broadcast(rb[:], rq[:])
                        for hh in range(2):
                            nc.vector.tensor_tensor(
                                ot_sb[ds(64 * hh, 64), pb, ds(qs, 512)],
                                oev[0:64, hh, :], rb[:, ds(512 * hh, 512)], MULT,
                            )
                # tail: last stripe's wo + anything left in the queue
                while drip:
                    pop_drip(1)
                for lq in range(12, 16):
                    for n in range(2):
                        wo_unit(lq, n, evac=nc.scalar.copy)

    nc.compile()
    return nc


_NC_CACHE = None


def _get_nc():
    global _NC_CACHE
    if _NC_CACHE is None:
        _NC_CACHE = _build_nc()
    return _NC_CACHE


def _shard_inputs(x, wq, wk, wv, wo, freqs_cos, freqs_sin, mask):
    """Host-side shard prep: pure layout/indexing transforms, no arithmetic."""
    f = np.float32
    perm = np.empty(64, np.int64)
    perm[:32] = 2 * np.arange(32)
    perm[32:] = 2 * np.arange(32) + 1

    x_t = np.ascontiguousarray(np.asarray(x, f).reshape(L, D).T)
    cosT = np.ascontiguousarray(np.asarray(freqs_cos, f).T)
    sinT = np.ascontiguousarray(np.asarray(freqs_sin, f).T)
    cos2 = np.ascontiguousarray(np.concatenate([cosT, cosT], 0))
    sin2 = np.ascontiguousarray(np.concatenate([sinT, sinT], 0))

    wq = np.asarray(wq, f)
    wk = np.asarray(wk, f)
    wv = np.asarray(wv, f)
    wo = np.asarray(wo, f)

    in_maps = []
    for i in range(NCORES):
        wq_i = wq[:, 4 * i * 64:(4 * i + 4) * 64]
        cols = []
        for pb in range(2):
            A = wq_i[:, (2 * pb) * 64:(2 * pb + 1) * 64][:, perm]
            Bc = wq_i[:, (2 * pb + 1) * 64:(2 * pb + 2) * 64][:, perm]
            cols.append(np.concatenate([A[:, :32], Bc[:, :32], A[:, 32:], Bc[:, 32:]], 1))
        wq_p = np.ascontiguousarray(np.concatenate(cols, 1))
        wk_p = wk[:, i * 64:(i + 1) * 64][:, perm]
        wv_i = wv[:, i * 64:(i + 1) * 64]
        wkv = np.ascontiguousarray(np.concatenate([wk_p, wv_i], 1))
        wo_i = np.ascontiguousarray(wo[4 * i * 64:(4 * i + 4) * 64, :])
        in_maps.append({
            "x_t": x_t, "wq_p": wq_p, "wkv": wkv, "wo_p": wo_i,
            "cos2": cos2, "sin2": sin2,
        })
    return in_maps


_last_results = None


def kernel(x, wq, wk, wv, wo, freqs_cos, freqs_sin, mask):
    global _last_results
    from concourse.bass_utils import run_bass_kernel_spmd

    nc = _get_nc()
    in_maps = _shard_inputs(x, wq, wk, wv, wo, freqs_cos, freqs_sin, mask)
    res = run_bass_kernel_spmd(nc, in_maps, core_ids=list(range(NCORES)))
    _last_results = res
    out = np.zeros((L, D), np.float32)
    for i in range(NCORES):
        out += np.asarray(res.results[i]["part"]).astype(np.float32)
    return out.reshape(B, L, D)
